# revision 2
# baseline (speedup 1.0000x reference)
"""ContextualLoss (CoCosNet, forward direction) Trainium2 kernel.

Shapes (hardcoded): X_features, Y_features [4, 256, 64, 64] f32 -> loss [4] f32.

Sharding: batch b on core pair (2b, 2b+1); each core handles 2048 of the 4096
rows (i) of its batch's [4096, 4096] similarity matrix and all 4096 columns
(j).  Row-wise reductions (min/sum/max over j) then need no cross-core
communication.  Each core emits one scalar: sum_i max_j A_ij over its rows.
Host combines: loss[b] = -log((s_2b + s_2b+1) / 4096).

Math performed per core (algebraically identical to the reference):
  mu_c      = mean_n Y[c, n]
  Xc=X-mu, Yc=Y-mu ; Xh=Xc/||Xc||, Yh=Yc/||Yc|| (per position, over channels)
  Sn[i, j]  = <Xh_i, Yh_j>                  (PE, f32r)
  M_i       = max_j Sn                      (DVE)
  m_i       = 1+1e-5 - M_i ; rm = 1/m_i
  w_ij      = exp(Sn*rm/H + (m-1)*rm/H)     (== exp((1 - d/m)/H), ACT)
  maxA_i    = exp(1e-5*rm/H) / sum_j w_ij
Column norms 1/||.|| are computed as exp(-0.5*ln(nsq)) on ACT from
partition-sum-broadcast PSUM produced by an all-ones [128,128] f32r matmul.
"""
import numpy as np

import concourse.bacc as bacc
import concourse.tile as tile
from concourse import mybir, bass_isa
from concourse.bass_utils import run_bass_kernel_spmd

B, C, HH, WW = 4, 256, 64, 64
N = HH * WW          # 4096 spatial positions
NCORES = 8
I = N // 2           # rows per core: 2048
J = N                # cols per core
H = 0.1
JH = J // 2          # psum half width: 2048 (4 banks)
NCT = C // 128       # 2 channel tiles
NIT = I // 128       # 16 i-tiles
f32 = mybir.dt.float32
f32r = mybir.dt.float32r

_NC_CACHE = {}


def _build(dbg=False):
    Exp = mybir.ActivationFunctionType.Exp
    Ln = mybir.ActivationFunctionType.Ln
    Square = mybir.ActivationFunctionType.Square
    Copy = mybir.ActivationFunctionType.Copy
    X = mybir.AxisListType.X
    XY = mybir.AxisListType.XY

    nc = bacc.Bacc('TRN2', target_bir_lowering=False, debug=False)
    xh_d = nc.declare_dram_parameter('xh', [C, I], f32, isOutput=False)
    yf_d = nc.declare_dram_parameter('yf', [C, J], f32, isOutput=False)
    out_d = nc.declare_dram_parameter('out', [1, 1], f32, isOutput=True)

    with tile.TileContext(nc) as tc:
        with (
            tc.tile_pool(name='big', bufs=1) as big,
            tc.tile_pool(name='small', bufs=1) as small,
            tc.tile_pool(name='tiny', bufs=3) as tiny,
            tc.tile_pool(name='wpool', bufs=2) as wpool,
        ):
            ysb = big.tile([128, NCT, J], f32r, tag='ysb')
            xsb = big.tile([128, NCT, I], f32r, tag='xsb')
            for k in range(NCT):
                nc.sync.dma_start(out=ysb[:, k, :], in_=yf_d[k*128:(k+1)*128, :].bitcast(f32r))
                nc.sync.dma_start(out=xsb[:, k, :], in_=xh_d[k*128:(k+1)*128, :].bitcast(f32r))

            # -mu per channel: ACT copy+accum (per half) then combine, scale by -1/N
            sumYh = small.tile([128, NCT, 2], f32)
            for k in range(NCT):
                for hh in range(2):
                    s = wpool.tile([128, JH], f32, tag='w')
                    nc.scalar.activation(s[:], ysb[:, k, hh*JH:(hh+1)*JH],
                                         Copy, accum_out=sumYh[:, k, hh:hh+1])
            negmu = small.tile([128, NCT], f32)
            nc.vector.reduce_sum(negmu[:], sumYh[:], axis=mybir.AxisListType.X)
            nc.scalar.mul(negmu[:], negmu[:], -1.0 / J)

            # squares of centered values, fused: sq = Square(v + negmu)
            ysq = big.tile([128, NCT, J], f32r, tag='ysq')
            xsq = big.tile([128, NCT, I], f32r, tag='xsq')
            for k in range(NCT):
                nc.scalar.activation(ysq[:, k, :], ysb[:, k, :], Square,
                                     bias=negmu[:, k:k+1], scale=1.0)
                nc.scalar.activation(xsq[:, k, :], xsb[:, k, :], Square,
                                     bias=negmu[:, k:k+1], scale=1.0)

            # center X, Y in place on gpsimd (runs parallel with ACT squares)
            for k in range(NCT):
                nc.gpsimd.tensor_scalar_add(ysb[:, k, :], ysb[:, k, :], negmu[:, k:k+1])
                nc.gpsimd.tensor_scalar_add(xsb[:, k, :], xsb[:, k, :], negmu[:, k:k+1])

            ones_f = small.tile([128, 128], f32)
            nc.vector.memset(ones_f[:], 1.0)
            ones = small.tile([128, 128], f32r)
            nc.scalar.copy(ones[:], ones_f[:])

            a_b = big.tile([128, I], f32, tag='ab')
            b_b = big.tile([128, J], f32, tag='bb')
            with tc.tile_pool(name='npsum', bufs=1, space='PSUM') as npsum:
                # nsqX broadcast over partitions -> a_b = exp(-0.5*ln(nsq))
                nx_ps = npsum.tile([128, I], f32, tag='nsq')
                for ib in range(I // 512):
                    for k in range(NCT):
                        nc.tensor.matmul(nx_ps[:, ib*512:(ib+1)*512], ones[:],
                                         xsq[:, k, ib*512:(ib+1)*512],
                                         start=(k == 0), stop=(k == NCT-1))
                nc.scalar.activation(a_b[:], nx_ps[:], Ln)
                nc.scalar.activation(a_b[:], a_b[:], Exp, scale=-0.5)

                ny_ps = npsum.tile([128, J], f32, tag='nsq')
                for jb in range(J // 512):
                    for k in range(NCT):
                        nc.tensor.matmul(ny_ps[:, jb*512:(jb+1)*512], ones[:],
                                         ysq[:, k, jb*512:(jb+1)*512],
                                         start=(k == 0), stop=(k == NCT-1))
                nc.scalar.activation(b_b[:], ny_ps[:], Ln)
                nc.scalar.activation(b_b[:], b_b[:], Exp, scale=-0.5)

            # normalize in place
            for k in range(NCT):
                nc.vector.tensor_mul(ysb[:, k, :], ysb[:, k, :], b_b[:])
                nc.vector.tensor_mul(xsb[:, k, :], xsb[:, k, :], a_b[:])

            stats = small.tile([128, NIT], f32)

            with tc.tile_pool(name='ppool', bufs=2, space='PSUM') as ppool:
                for t in range(NIT):
                    snA = ppool.tile([128, JH], f32, tag='sn')
                    snB = ppool.tile([128, JH], f32, tag='sn')
                    for h, sn in enumerate((snA, snB)):
                        for jb in range(JH // 512):
                            for k in range(NCT):
                                nc.tensor.matmul(
                                    sn[:, jb*512:(jb+1)*512],
                                    xsb[:, k, t*128:(t+1)*128],
                                    ysb[:, k, h*JH + jb*512: h*JH + (jb+1)*512],
                                    start=(k == 0), stop=(k == NCT-1))
                    MA = tiny.tile([128, 1], f32, tag='MA')
                    MB = tiny.tile([128, 1], f32, tag='MB')
                    nc.vector.reduce_max(MA[:], snA.rearrange('p (b j) -> p b j', j=512), axis=XY)
                    nc.vector.reduce_max(MB[:], snB.rearrange('p (b j) -> p b j', j=512), axis=XY)
                    M = tiny.tile([128, 1], f32, tag='M')
                    nc.vector.tensor_max(M[:], MA[:], MB[:])
                    # q = (1+1e-5) - M ; rm = 1/q
                    q = tiny.tile([128, 1], f32, tag='q')
                    nc.vector.tensor_scalar(q[:], M[:], -1.0, 1.0 + 1e-5,
                                            op0=mybir.AluOpType.mult, op1=mybir.AluOpType.add)
                    rm = tiny.tile([128, 1], f32, tag='rm')
                    nc.vector.reciprocal(rm[:], q[:])
                    s10 = tiny.tile([128, 1], f32, tag='s10')
                    nc.vector.tensor_scalar(s10[:], rm[:], 1.0 / H, None, op0=mybir.AluOpType.mult)
                    u = tiny.tile([128, 1], f32, tag='u')
                    nc.vector.tensor_scalar(u[:], q[:], 1.0, 1.0 / H,
                                            op0=mybir.AluOpType.subtract, op1=mybir.AluOpType.mult)
                    be = tiny.tile([128, 1], f32, tag='be')
                    nc.vector.tensor_mul(be[:], u[:], rm[:])
                    swA = tiny.tile([128, 1], f32, tag='swA')
                    swB = tiny.tile([128, 1], f32, tag='swB')
                    wA = wpool.tile([128, JH], f32, tag='w')
                    wB = wpool.tile([128, JH], f32, tag='w')
                    nc.scalar.activation(wA[:], snA[:], Exp, bias=be[:], scale=s10[:],
                                         accum_out=swA[:])
                    nc.scalar.activation(wB[:], snB[:], Exp, bias=be[:], scale=s10[:],
                                         accum_out=swB[:])
                    sumw = tiny.tile([128, 1], f32, tag='sumw')
                    nc.vector.tensor_add(sumw[:], swA[:], swB[:])
                    maxw = tiny.tile([128, 1], f32, tag='maxw')
                    nc.scalar.activation(maxw[:], rm[:], Exp, bias=0.0, scale=1e-5 / H)
                    rs = tiny.tile([128, 1], f32, tag='rs')
                    nc.vector.reciprocal(rs[:], sumw[:])
                    nc.vector.tensor_mul(stats[:, t:t+1], maxw[:], rs[:])

            # partition-sum of stats, then free-dim sum -> scalar
            pstats = small.tile([128, NIT], f32)
            nc.gpsimd.partition_all_reduce(pstats[:], stats[:], channels=128,
                                           reduce_op=bass_isa.ReduceOp.add)
            fin = small.tile([1, 1], f32)
            nc.vector.reduce_sum(fin[:], pstats[0:1, :], axis=X)
            nc.sync.dma_start(out=out_d[:], in_=fin[:])
    nc.compile()
    return nc


def get_nc():
    if 'nc' not in _NC_CACHE:
        _NC_CACHE['nc'] = _build()
    return _NC_CACHE['nc']


def kernel(X_features: np.ndarray, Y_features: np.ndarray, trace: bool = False):
    X = np.ascontiguousarray(np.asarray(X_features, dtype=np.float32)).reshape(B, C, N)
    Y = np.ascontiguousarray(np.asarray(Y_features, dtype=np.float32)).reshape(B, C, N)
    in_maps = []
    for c in range(NCORES):
        b, half = c // 2, c % 2
        in_maps.append({
            'xh': np.ascontiguousarray(X[b][:, half*I:(half+1)*I]),
            'yf': np.ascontiguousarray(Y[b]),
        })
    nc = get_nc()
    res = run_bass_kernel_spmd(nc, in_maps, list(range(NCORES)), trace=trace)
    s = np.array([res.results[c]['out'][0, 0] for c in range(NCORES)], dtype=np.float64)
    cx = (s[0::2] + s[1::2]) / N
    loss = (-np.log(cx)).astype(np.float32)
    if trace:
        return loss, res
    return loss


# revision 5
# speedup vs baseline: 1.2858x; 1.2858x over previous
"""ContextualLoss (CoCosNet, forward direction) Trainium2 kernel.

Shapes (hardcoded): X_features, Y_features [4, 256, 64, 64] f32 -> loss [4] f32.

Sharding: batch b on core pair (2b, 2b+1); each core handles 2048 of the 4096
rows (i) of its batch's [4096, 4096] similarity matrix and all 4096 columns
(j).  Row-wise reductions (min/sum/max over j) then need no cross-core
communication.  Each core emits one scalar: sum_i max_j A_ij over its rows.
Host combines: loss[b] = -log((s_2b + s_2b+1) / 4096).

Math performed per core (algebraically identical to the reference):
  mu_c      = mean_n Y[c, n]
  Xc=X-mu, Yc=Y-mu ; Xh=Xc/||Xc||, Yh=Yc/||Yc|| (per position, over channels)
  Sn[i, j]  = <Xh_i, Yh_j>                  (PE, f32r)
  M_i       = max_j Sn                      (DVE)
  m_i       = 1+1e-5 - M_i ; rm = 1/m_i
  w_ij      = exp(Sn*rm/H + (m-1)*rm/H)     (== exp((1 - d/m)/H), ACT)
  maxA_i    = exp(1e-5*rm/H) / sum_j w_ij
Column norms 1/||.|| are computed as exp(-0.5*ln(nsq)) on ACT from
partition-sum-broadcast PSUM produced by an all-ones [128,128] f32r matmul.
"""
import numpy as np

import concourse.bacc as bacc
import concourse.tile as tile
from concourse import mybir, bass_isa
from concourse.bass_utils import run_bass_kernel_spmd

B, C, HH, WW = 4, 256, 64, 64
N = HH * WW          # 4096 spatial positions
NCORES = 8
I = N // 2           # rows per core: 2048
J = N                # cols per core
H = 0.1
JH = J // 2          # psum half width: 2048 (4 banks)
NCT = C // 128       # 2 channel tiles
NIT = I // 128       # 16 i-tiles
f32 = mybir.dt.float32
f32r = mybir.dt.float32r

_NC_CACHE = {}


def _build(dbg=False):
    Exp = mybir.ActivationFunctionType.Exp
    Ln = mybir.ActivationFunctionType.Ln
    Square = mybir.ActivationFunctionType.Square
    Copy = mybir.ActivationFunctionType.Copy
    X = mybir.AxisListType.X
    XY = mybir.AxisListType.XY

    nc = bacc.Bacc('TRN2', target_bir_lowering=False, debug=False)
    xh_d = nc.declare_dram_parameter('xh', [C, I], f32, isOutput=False)
    yf_d = nc.declare_dram_parameter('yf', [C, J], f32, isOutput=False)
    out_d = nc.declare_dram_parameter('out', [1, 1], f32, isOutput=True)

    with tile.TileContext(nc) as tc:
        with (
            tc.tile_pool(name='big', bufs=1) as big,
            tc.tile_pool(name='small', bufs=1) as small,
            tc.tile_pool(name='tiny', bufs=3) as tiny,
            tc.tile_pool(name='wpool', bufs=2) as wpool,
        ):
            ysb = big.tile([128, NCT, J], f32r, tag='ysb')
            xsb = big.tile([128, NCT, I], f32r, tag='xsb')
            for k in range(NCT):
                nc.sync.dma_start(out=ysb[:, k, :], in_=yf_d[k*128:(k+1)*128, :].bitcast(f32r))
                nc.sync.dma_start(out=xsb[:, k, :], in_=xh_d[k*128:(k+1)*128, :].bitcast(f32r))

            # -mu per channel: ACT copy+accum (per half) then combine, scale by -1/N
            sumYh = small.tile([128, NCT, 2], f32)
            for k in range(NCT):
                for hh in range(2):
                    s = wpool.tile([128, JH], f32, tag='w')
                    nc.scalar.activation(s[:], ysb[:, k, hh*JH:(hh+1)*JH],
                                         Copy, accum_out=sumYh[:, k, hh:hh+1])
            negmu = small.tile([128, NCT], f32)
            nc.vector.reduce_sum(negmu[:], sumYh[:], axis=mybir.AxisListType.X)
            nc.scalar.mul(negmu[:], negmu[:], -1.0 / J)

            # squares of centered values, fused: sq = Square(v + negmu)
            ysq = big.tile([128, NCT, J], f32r, tag='ysq')
            xsq = big.tile([128, NCT, I], f32r, tag='xsq')
            for k in range(NCT):
                nc.scalar.activation(ysq[:, k, :], ysb[:, k, :], Square,
                                     bias=negmu[:, k:k+1], scale=1.0)
                nc.scalar.activation(xsq[:, k, :], xsb[:, k, :], Square,
                                     bias=negmu[:, k:k+1], scale=1.0)

            # center X, Y in place on gpsimd (runs parallel with ACT squares)
            for k in range(NCT):
                nc.gpsimd.tensor_scalar_add(ysb[:, k, :], ysb[:, k, :], negmu[:, k:k+1])
                nc.gpsimd.tensor_scalar_add(xsb[:, k, :], xsb[:, k, :], negmu[:, k:k+1])

            ones_f = small.tile([128, 128], f32)
            nc.vector.memset(ones_f[:], 1.0)
            ones = small.tile([128, 128], f32r)
            nc.scalar.copy(ones[:], ones_f[:])

            a_b = big.tile([128, I], f32, tag='ab')
            b_b = big.tile([128, J], f32, tag='bb')
            with tc.tile_pool(name='npsum', bufs=1, space='PSUM') as npsum:
                # nsqX broadcast over partitions -> a_b = exp(-0.5*ln(nsq))
                nx_ps = npsum.tile([128, I], f32, tag='nsq')
                for ib in range(I // 512):
                    for k in range(NCT):
                        nc.tensor.matmul(nx_ps[:, ib*512:(ib+1)*512], ones[:],
                                         xsq[:, k, ib*512:(ib+1)*512],
                                         start=(k == 0), stop=(k == NCT-1))
                nc.scalar.activation(a_b[:], nx_ps[:], Ln)
                nc.scalar.activation(a_b[:], a_b[:], Exp, scale=-0.5)

                ny_ps = npsum.tile([128, J], f32, tag='nsq')
                for jb in range(J // 512):
                    for k in range(NCT):
                        nc.tensor.matmul(ny_ps[:, jb*512:(jb+1)*512], ones[:],
                                         ysq[:, k, jb*512:(jb+1)*512],
                                         start=(k == 0), stop=(k == NCT-1))
                nc.scalar.activation(b_b[:], ny_ps[:], Ln)
                nc.scalar.activation(b_b[:], b_b[:], Exp, scale=-0.5)

            # normalize in place
            for k in range(NCT):
                nc.vector.tensor_mul(ysb[:, k, :], ysb[:, k, :], b_b[:])
                nc.vector.tensor_mul(xsb[:, k, :], xsb[:, k, :], a_b[:])

            stats = small.tile([128, NIT], f32)

            QW = J // 4  # 1024: quarter width, 2 PSUM banks
            Identity = mybir.ActivationFunctionType.Identity
            with tc.tile_pool(name='ppool', bufs=4, space='PSUM') as ppool:
                for t in range(NIT):
                    # Sn quarters; bufs=4 so exp(t) frees 2-bank chunks
                    # progressively and PE starts tile t+1 behind ACT.
                    sns = [ppool.tile([128, QW], f32, tag='sn', name=f'sn_{t}_{qi}')
                           for qi in range(4)]
                    MQ = tiny.tile([128, 4], f32, tag='MQ')
                    for qi, sn in enumerate(sns):
                        for jb in range(QW // 512):
                            for k in range(NCT):
                                nc.tensor.matmul(
                                    sn[:, jb*512:(jb+1)*512],
                                    xsb[:, k, t*128:(t+1)*128],
                                    ysb[:, k, qi*QW + jb*512: qi*QW + (jb+1)*512],
                                    start=(k == 0), stop=(k == NCT-1))
                        nc.vector.reduce_max(MQ[:, qi:qi+1],
                                             sn.rearrange('p (b j) -> p b j', j=512), axis=XY)
                    M = tiny.tile([128, 1], f32, tag='M')
                    nc.vector.reduce_max(M[:], MQ[:], axis=X)
                    # q = (1+1e-5) - M ; rm = 1/q ; s10 = rm/H ; be = (q-1)*rm/H
                    q = tiny.tile([128, 1], f32, tag='q')
                    nc.vector.tensor_scalar(q[:], M[:], -1.0, 1.0 + 1e-5,
                                            op0=mybir.AluOpType.mult, op1=mybir.AluOpType.add)
                    rm = tiny.tile([128, 1], f32, tag='rm')
                    nc.vector.reciprocal(rm[:], q[:])
                    s10 = tiny.tile([128, 1], f32, tag='s10')
                    nc.scalar.activation(s10[:], rm[:], Copy, scale=1.0 / H)
                    u = tiny.tile([128, 1], f32, tag='u')
                    nc.vector.tensor_scalar(u[:], q[:], 1.0, 1.0 / H,
                                            op0=mybir.AluOpType.subtract, op1=mybir.AluOpType.mult)
                    be = tiny.tile([128, 1], f32, tag='be')
                    nc.vector.tensor_mul(be[:], u[:], rm[:])
                    sums4 = tiny.tile([128, 4], f32, tag='sums4')
                    for qi, sn in enumerate(sns):
                        wq = wpool.tile([128, QW], f32, tag='w')
                        nc.scalar.activation(wq[:], sn[:], Exp, bias=be[:], scale=s10[:],
                                             accum_out=sums4[:, qi:qi+1])
                    sumw = tiny.tile([128, 1], f32, tag='sumw')
                    nc.vector.reduce_sum(sumw[:], sums4[:], axis=X)
                    maxw = tiny.tile([128, 1], f32, tag='maxw')
                    nc.scalar.activation(maxw[:], rm[:], Exp, bias=0.0, scale=1e-5 / H)
                    rs = tiny.tile([128, 1], f32, tag='rs')
                    nc.vector.reciprocal(rs[:], sumw[:])
                    nc.vector.tensor_mul(stats[:, t:t+1], maxw[:], rs[:])

            # partition-sum of stats, then free-dim sum -> scalar
            pstats = small.tile([128, NIT], f32)
            nc.gpsimd.partition_all_reduce(pstats[:], stats[:], channels=128,
                                           reduce_op=bass_isa.ReduceOp.add)
            fin = small.tile([1, 1], f32)
            nc.vector.reduce_sum(fin[:], pstats[0:1, :], axis=X)
            nc.sync.dma_start(out=out_d[:], in_=fin[:])
    nc.compile()
    return nc


def get_nc():
    if 'nc' not in _NC_CACHE:
        _NC_CACHE['nc'] = _build()
    return _NC_CACHE['nc']


def kernel(X_features: np.ndarray, Y_features: np.ndarray, trace: bool = False):
    X = np.ascontiguousarray(np.asarray(X_features, dtype=np.float32)).reshape(B, C, N)
    Y = np.ascontiguousarray(np.asarray(Y_features, dtype=np.float32)).reshape(B, C, N)
    in_maps = []
    for c in range(NCORES):
        b, half = c // 2, c % 2
        in_maps.append({
            'xh': np.ascontiguousarray(X[b][:, half*I:(half+1)*I]),
            'yf': np.ascontiguousarray(Y[b]),
        })
    nc = get_nc()
    res = run_bass_kernel_spmd(nc, in_maps, list(range(NCORES)), trace=trace)
    s = np.array([res.results[c]['out'][0, 0] for c in range(NCORES)], dtype=np.float64)
    cx = (s[0::2] + s[1::2]) / N
    loss = (-np.log(cx)).astype(np.float32)
    if trace:
        return loss, res
    return loss


# revision 6
# speedup vs baseline: 1.3046x; 1.0146x over previous
"""ContextualLoss (CoCosNet, forward direction) Trainium2 kernel.

Shapes (hardcoded): X_features, Y_features [4, 256, 64, 64] f32 -> loss [4] f32.

Sharding: batch b on core pair (2b, 2b+1); each core handles 2048 of the 4096
rows (i) of its batch's [4096, 4096] similarity matrix and all 4096 columns
(j).  Row-wise reductions (min/sum/max over j) then need no cross-core
communication.  Each core emits one scalar: sum_i max_j A_ij over its rows.
Host combines: loss[b] = -log((s_2b + s_2b+1) / 4096).

Math performed per core (algebraically identical to the reference):
  mu_c      = mean_n Y[c, n]
  Xc=X-mu, Yc=Y-mu ; Xh=Xc/||Xc||, Yh=Yc/||Yc|| (per position, over channels)
  Sn[i, j]  = <Xh_i, Yh_j>                  (PE, f32r)
  M_i       = max_j Sn                      (DVE)
  m_i       = 1+1e-5 - M_i ; rm = 1/m_i
  w_ij      = exp(Sn*rm/H + (m-1)*rm/H)     (== exp((1 - d/m)/H), ACT)
  maxA_i    = exp(1e-5*rm/H) / sum_j w_ij
Column norms 1/||.|| are computed as exp(-0.5*ln(nsq)) on ACT from
partition-sum-broadcast PSUM produced by an all-ones [128,128] f32r matmul.
"""
import numpy as np

import concourse.bacc as bacc
import concourse.tile as tile
from concourse import mybir, bass_isa
from concourse.bass_utils import run_bass_kernel_spmd

B, C, HH, WW = 4, 256, 64, 64
N = HH * WW          # 4096 spatial positions
NCORES = 8
I = N // 2           # rows per core: 2048
J = N                # cols per core
H = 0.1
JH = J // 2          # psum half width: 2048 (4 banks)
NCT = C // 128       # 2 channel tiles
NIT = I // 128       # 16 i-tiles
f32 = mybir.dt.float32
f32r = mybir.dt.float32r

_NC_CACHE = {}


def _build(dbg=False):
    Exp = mybir.ActivationFunctionType.Exp
    Ln = mybir.ActivationFunctionType.Ln
    Square = mybir.ActivationFunctionType.Square
    Copy = mybir.ActivationFunctionType.Copy
    X = mybir.AxisListType.X
    XY = mybir.AxisListType.XY

    nc = bacc.Bacc('TRN2', target_bir_lowering=False, debug=False)
    xh_d = nc.declare_dram_parameter('xh', [C, I], f32, isOutput=False)
    yf_d = nc.declare_dram_parameter('yf', [C, J], f32, isOutput=False)
    out_d = nc.declare_dram_parameter('out', [1, 1], f32, isOutput=True)

    with tile.TileContext(nc) as tc:
        with (
            tc.tile_pool(name='big', bufs=1) as big,
            tc.tile_pool(name='small', bufs=1) as small,
            tc.tile_pool(name='tiny', bufs=4) as tiny,
            tc.tile_pool(name='wpool', bufs=4) as wpool,
        ):
            ysb = big.tile([128, NCT, J], f32r, tag='ysb')
            xsb = big.tile([128, NCT, I], f32r, tag='xsb')
            for k in range(NCT):
                nc.sync.dma_start(out=ysb[:, k, :], in_=yf_d[k*128:(k+1)*128, :].bitcast(f32r))
                nc.sync.dma_start(out=xsb[:, k, :], in_=xh_d[k*128:(k+1)*128, :].bitcast(f32r))

            # -mu per channel: ACT copy+accum (per half) then combine, scale by -1/N
            sumYh = small.tile([128, NCT, 2], f32)
            for k in range(NCT):
                for hh in range(2):
                    s = wpool.tile([128, JH], f32, tag='w')
                    nc.scalar.activation(s[:], ysb[:, k, hh*JH:(hh+1)*JH],
                                         Copy, accum_out=sumYh[:, k, hh:hh+1])
            negmu = small.tile([128, NCT], f32)
            nc.vector.reduce_sum(negmu[:], sumYh[:], axis=mybir.AxisListType.X)
            nc.scalar.mul(negmu[:], negmu[:], -1.0 / J)

            # squares of centered values, fused: sq = Square(v + negmu)
            ysq = big.tile([128, NCT, J], f32r, tag='ysq')
            xsq = big.tile([128, NCT, I], f32r, tag='xsq')
            for k in range(NCT):
                nc.scalar.activation(ysq[:, k, :], ysb[:, k, :], Square,
                                     bias=negmu[:, k:k+1], scale=1.0)
                nc.scalar.activation(xsq[:, k, :], xsb[:, k, :], Square,
                                     bias=negmu[:, k:k+1], scale=1.0)

            # center X, Y in place on gpsimd (runs parallel with ACT squares)
            for k in range(NCT):
                nc.gpsimd.tensor_scalar_add(ysb[:, k, :], ysb[:, k, :], negmu[:, k:k+1])
                nc.gpsimd.tensor_scalar_add(xsb[:, k, :], xsb[:, k, :], negmu[:, k:k+1])

            ones_f = small.tile([128, 128], f32)
            nc.vector.memset(ones_f[:], 1.0)
            ones = small.tile([128, 128], f32r)
            nc.scalar.copy(ones[:], ones_f[:])

            a_b = big.tile([128, I], f32, tag='ab')
            b_b = big.tile([128, J], f32, tag='bb')
            with tc.tile_pool(name='npsum', bufs=1, space='PSUM') as npsum:
                # nsqX broadcast over partitions -> a_b = exp(-0.5*ln(nsq))
                nx_ps = npsum.tile([128, I], f32, tag='nsq')
                for ib in range(I // 512):
                    for k in range(NCT):
                        nc.tensor.matmul(nx_ps[:, ib*512:(ib+1)*512], ones[:],
                                         xsq[:, k, ib*512:(ib+1)*512],
                                         start=(k == 0), stop=(k == NCT-1))
                nc.scalar.activation(a_b[:], nx_ps[:], Ln)
                nc.scalar.activation(a_b[:], a_b[:], Exp, scale=-0.5)

                ny_ps = npsum.tile([128, J], f32, tag='nsq')
                for jb in range(J // 512):
                    for k in range(NCT):
                        nc.tensor.matmul(ny_ps[:, jb*512:(jb+1)*512], ones[:],
                                         ysq[:, k, jb*512:(jb+1)*512],
                                         start=(k == 0), stop=(k == NCT-1))
                nc.scalar.activation(b_b[:], ny_ps[:], Ln)
                nc.scalar.activation(b_b[:], b_b[:], Exp, scale=-0.5)

            # normalize in place, chunked so the first main matmuls start early
            CHK = J // 4
            for cq in range(4):
                sl = slice(cq*CHK, (cq+1)*CHK)
                for k in range(NCT):
                    nc.vector.tensor_mul(ysb[:, k, sl], ysb[:, k, sl], b_b[:, sl])
            XCHK = I // 4
            for cq in range(4):
                sl = slice(cq*XCHK, (cq+1)*XCHK)
                for k in range(NCT):
                    nc.vector.tensor_mul(xsb[:, k, sl], xsb[:, k, sl], a_b[:, sl])

            stats = small.tile([128, NIT], f32)

            QW = J // 4  # 1024: quarter width, 2 PSUM banks
            Identity = mybir.ActivationFunctionType.Identity
            with tc.tile_pool(name='ppool', bufs=4, space='PSUM') as ppool:
                for t in range(NIT):
                    # Sn quarters; bufs=4 so exp(t) frees 2-bank chunks
                    # progressively and PE starts tile t+1 behind ACT.
                    sns = [ppool.tile([128, QW], f32, tag='sn', name=f'sn_{t}_{qi}')
                           for qi in range(4)]
                    MQ = tiny.tile([128, 4], f32, tag='MQ')
                    for qi, sn in enumerate(sns):
                        for jb in range(QW // 512):
                            for k in range(NCT):
                                nc.tensor.matmul(
                                    sn[:, jb*512:(jb+1)*512],
                                    xsb[:, k, t*128:(t+1)*128],
                                    ysb[:, k, qi*QW + jb*512: qi*QW + (jb+1)*512],
                                    start=(k == 0), stop=(k == NCT-1))
                        nc.vector.reduce_max(MQ[:, qi:qi+1],
                                             sn.rearrange('p (b j) -> p b j', j=512), axis=XY)
                    M = tiny.tile([128, 1], f32, tag='M')
                    nc.vector.reduce_max(M[:], MQ[:], axis=X)
                    # q = (1+1e-5) - M ; rm = 1/q ; s10 = rm/H ; be = (q-1)*rm/H
                    q = tiny.tile([128, 1], f32, tag='q')
                    nc.vector.tensor_scalar(q[:], M[:], -1.0, 1.0 + 1e-5,
                                            op0=mybir.AluOpType.mult, op1=mybir.AluOpType.add)
                    rm = tiny.tile([128, 1], f32, tag='rm')
                    nc.vector.reciprocal(rm[:], q[:])
                    s10 = tiny.tile([128, 1], f32, tag='s10')
                    nc.scalar.activation(s10[:], rm[:], Copy, scale=1.0 / H)
                    u = tiny.tile([128, 1], f32, tag='u')
                    nc.vector.tensor_scalar(u[:], M[:], -1.0 / H, 1e-5 / H,
                                            op0=mybir.AluOpType.mult, op1=mybir.AluOpType.add)
                    be = tiny.tile([128, 1], f32, tag='be')
                    nc.vector.tensor_mul(be[:], u[:], rm[:])
                    sums4 = tiny.tile([128, 4], f32, tag='sums4')
                    for qi, sn in enumerate(sns):
                        wq = wpool.tile([128, QW], f32, tag='w')
                        nc.scalar.activation(wq[:], sn[:], Exp, bias=be[:], scale=s10[:],
                                             accum_out=sums4[:, qi:qi+1])
                    sumw = tiny.tile([128, 1], f32, tag='sumw')
                    nc.vector.reduce_sum(sumw[:], sums4[:], axis=X)
                    maxw = tiny.tile([128, 1], f32, tag='maxw')
                    nc.scalar.activation(maxw[:], rm[:], Exp, bias=0.0, scale=1e-5 / H)
                    rs = tiny.tile([128, 1], f32, tag='rs')
                    nc.vector.reciprocal(rs[:], sumw[:])
                    nc.vector.tensor_mul(stats[:, t:t+1], maxw[:], rs[:])

            # partition-sum of stats, then free-dim sum -> scalar
            pstats = small.tile([128, NIT], f32)
            nc.gpsimd.partition_all_reduce(pstats[:], stats[:], channels=128,
                                           reduce_op=bass_isa.ReduceOp.add)
            fin = small.tile([1, 1], f32)
            nc.vector.reduce_sum(fin[:], pstats[0:1, :], axis=X)
            nc.sync.dma_start(out=out_d[:], in_=fin[:])
    nc.compile()
    return nc


def get_nc():
    if 'nc' not in _NC_CACHE:
        _NC_CACHE['nc'] = _build()
    return _NC_CACHE['nc']


def kernel(X_features: np.ndarray, Y_features: np.ndarray, trace: bool = False):
    X = np.ascontiguousarray(np.asarray(X_features, dtype=np.float32)).reshape(B, C, N)
    Y = np.ascontiguousarray(np.asarray(Y_features, dtype=np.float32)).reshape(B, C, N)
    in_maps = []
    for c in range(NCORES):
        b, half = c // 2, c % 2
        in_maps.append({
            'xh': np.ascontiguousarray(X[b][:, half*I:(half+1)*I]),
            'yf': np.ascontiguousarray(Y[b]),
        })
    nc = get_nc()
    res = run_bass_kernel_spmd(nc, in_maps, list(range(NCORES)), trace=trace)
    s = np.array([res.results[c]['out'][0, 0] for c in range(NCORES)], dtype=np.float64)
    cx = (s[0::2] + s[1::2]) / N
    loss = (-np.log(cx)).astype(np.float32)
    if trace:
        return loss, res
    return loss
